# revision 3
# baseline (speedup 1.0000x reference)
# VQ codebook layer for Trainium2 (8 NeuronCores, token-sharded).
#
# Per core (1024 tokens): stream the transposed codebook through SBUF in
# 2048-code chunks, compute coarse scores 2*x.c - c2 on the tensor engine in
# float32r (tf32-class, 4x faster than fp32; the c2 term rides along as a 9th
# contraction sub-chunk), take the per-chunk top-8 with the DVE max/max_index
# ops, and keep a running top-12 candidate list per token.  A candidate with
# true global rank <= 8 always has chunk rank <= 8 (pigeonhole), so chunk
# top-8 + running top-12 only loses true top-8 members when coarse error
# exceeds the rank-8..12 gap - negligible.  Then re-score the 12 candidates
# exactly in fp32 (indirect-DMA gather of augmented codebook rows + DVE
# multiply/reduce), pick the exact top-8, and gather + average the winning
# rows for the output.
import numpy as np

_B, _S, _D = 4, 2048, 1024
_C = 16384
_K = 8
_NCORES = 8
_TPC = (_B * _S) // _NCORES  # tokens per core
_P = 128
_CHUNK = 2048
_KSUB = _D // _P + 1  # 8 D sub-chunks + 1 aug (c2) sub-chunk
_AUGW = 1032  # gathered row: 1024 codebook + c2 + pad
_M = 12  # rescue candidate count
_COMB = 20  # merge width: 12 running + 8 new
_NEG = -1.0e30

_cache = {}


def _build(tt_n, cc_n):
    """Build the per-core Bass program. tt_n token tiles, cc_n code chunks."""
    import concourse.bass as bass
    import concourse.tile as tile
    from concourse import bacc, mybir

    f32 = mybir.dt.float32
    f32r = mybir.dt.float32r
    u32 = mybir.dt.uint32
    i32 = mybir.dt.int32
    AX = mybir.AxisListType.X
    EQ = mybir.AluOpType.is_equal
    MUL = mybir.AluOpType.mult

    ncodes = cc_n * _CHUNK
    ntok = tt_n * _P

    nc = bacc.Bacc("TRN2", target_bir_lowering=False, debug=False)
    xt = nc.declare_dram_parameter("xt", [_P, _KSUB, ntok], f32r, isOutput=False)
    ct = nc.declare_dram_parameter("ct", [cc_n, _P, _KSUB, _CHUNK], f32r, isOutput=False)
    cbaug = nc.declare_dram_parameter("cbaug", [ncodes, _AUGW], f32, isOutput=False)
    xaug = nc.declare_dram_parameter("xaug", [_P, tt_n, _AUGW], f32, isOutput=False)
    iota = nc.declare_dram_parameter("iota", [_P, 32], f32, isOutput=False)
    out_x = nc.declare_dram_parameter("out_x", [ntok, _D], f32, isOutput=True)
    out_ids = nc.declare_dram_parameter("out_ids", [ntok, _K], i32, isOutput=True)

    with tile.TileContext(nc) as tc:
        with (
            tc.tile_pool(name="const", bufs=1) as constp,
            tc.tile_pool(name="cand", bufs=1) as candp,
        ):
            iota_t = constp.tile([_P, 32], f32)
            nc.sync.dma_start(iota_t[:], iota[:])
            xaug_t = constp.tile([_P, tt_n, _AUGW], f32)
            nc.sync.dma_start(xaug_t[:], xaug[:])

            cands_v = candp.tile([_P, tt_n, _COMB], f32)
            cands_i = candp.tile([_P, tt_n, _COMB], f32)
            nc.vector.memset(cands_v[:], _NEG)
            nc.vector.memset(cands_i[:], 0.0)

            # ---------------- Phase A: coarse scores + running top-12 ----
            with (
                tc.tile_pool(name="xtp", bufs=1) as xtp,
                tc.tile_pool(name="ctp", bufs=12) as ctp,
                tc.tile_pool(name="scr", bufs=3) as scr,
                tc.tile_pool(name="psum", bufs=2, space="PSUM") as psp,
            ):
                xt_t = xtp.tile([_P, _KSUB, ntok], f32r)
                nc.sync.dma_start(xt_t[:], xt[:])

                for cc in range(cc_n):
                    subs = []
                    for s in range(_KSUB):
                        st = ctp.tile([_P, _CHUNK], f32r, tag="ct")
                        nc.sync.dma_start(st[:], ct[cc, :, s, :])
                        subs.append(st)
                    for tt in range(tt_n):
                        ps = psp.tile([_P, _CHUNK], f32, tag="ps")
                        for q in range(_CHUNK // 512):
                            qs = slice(q * 512, (q + 1) * 512)
                            for s in range(_KSUB):
                                nc.tensor.matmul(
                                    ps[:, qs],
                                    xt_t[:, s, tt * _P : (tt + 1) * _P],
                                    subs[s][:, qs],
                                    start=(s == 0),
                                    stop=(s == _KSUB - 1),
                                )
                        # chunk top-8 (coarse)
                        v8 = scr.tile([_P, 8], f32, tag="v8")
                        nc.vector.max(out=v8[:], in_=ps[:])
                        p8 = scr.tile([_P, 8], u32, tag="p8")
                        nc.vector.max_index(p8[:], v8[:], ps[:])
                        p8f = scr.tile([_P, 8], f32, tag="p8f")
                        nc.vector.tensor_copy(p8f[:], p8[:])
                        # append to slots 12:20 of the running list
                        nc.vector.tensor_scalar_add(
                            cands_i[:, tt, _M:_COMB], p8f[:], float(cc * _CHUNK)
                        )
                        nc.vector.tensor_copy(cands_v[:, tt, _M:_COMB], v8[:])
                        # merge: top-12 of 20
                        comb_v = cands_v[:, tt, :]
                        m8v = scr.tile([_P, 8], f32, tag="m8v")
                        nc.vector.max(out=m8v[:], in_=comb_v)
                        m8p = scr.tile([_P, 8], u32, tag="m8p")
                        nc.vector.max_index(m8p[:], m8v[:], comb_v)
                        rep = scr.tile([_P, _COMB], f32, tag="rep")
                        nc.vector.match_replace(
                            out=rep[:], in_to_replace=m8v[:], in_values=comb_v,
                            imm_value=_NEG,
                        )
                        m4v = scr.tile([_P, 8], f32, tag="m4v")
                        nc.vector.max(out=m4v[:], in_=rep[:])
                        m4p = scr.tile([_P, 8], u32, tag="m4p")
                        nc.vector.max_index(m4p[:], m4v[:], rep[:])
                        pos = scr.tile([_P, _M], f32, tag="pos")
                        nc.vector.tensor_copy(pos[:, 0:8], m8p[:])
                        nc.vector.tensor_copy(pos[:, 8:_M], m4p[:, 0 : _M - 8])
                        sel = scr.tile([_P, _M, _COMB], f32, tag="sel")
                        nc.vector.tensor_tensor(
                            out=sel[:],
                            in0=pos[:, :, None].to_broadcast([_P, _M, _COMB]),
                            in1=iota_t[:, None, 0:_COMB].to_broadcast([_P, _M, _COMB]),
                            op=EQ,
                        )
                        nc.vector.tensor_tensor(
                            out=sel[:],
                            in0=sel[:],
                            in1=cands_i[:, tt, None, :].to_broadcast([_P, _M, _COMB]),
                            op=MUL,
                        )
                        newi = scr.tile([_P, _M], f32, tag="newi")
                        nc.vector.reduce_sum(newi[:], sel[:], axis=AX)
                        nc.vector.tensor_copy(cands_v[:, tt, 0:8], m8v[:])
                        nc.vector.tensor_copy(cands_v[:, tt, 8:_M], m4v[:, 0 : _M - 8])
                        nc.vector.tensor_copy(cands_i[:, tt, 0:_M], newi[:])

            # ---------------- Phase B: exact rescore + gather ------------
            with tc.tile_pool(name="phb", bufs=2) as pb:
                for tt in range(tt_n):
                    idx12 = pb.tile([_P, _M], u32, tag="idx12")
                    nc.vector.tensor_copy(idx12[:], cands_i[:, tt, 0:_M])
                    G = pb.tile([_P, _M, _AUGW], f32, tag="G")
                    for j in range(_M):
                        nc.gpsimd.indirect_dma_start(
                            out=G[:, j, :],
                            out_offset=None,
                            in_=cbaug[:],
                            in_offset=bass.IndirectOffsetOnAxis(
                                ap=idx12[:, j : j + 1], axis=0
                            ),
                        )
                    nc.vector.tensor_tensor(
                        out=G[:],
                        in0=G[:],
                        in1=xaug_t[:, tt, None, :].to_broadcast([_P, _M, _AUGW]),
                        op=MUL,
                    )
                    rsc = pb.tile([_P, _M], f32, tag="rsc")
                    nc.vector.reduce_sum(rsc[:], G[:], axis=AX)
                    v8 = pb.tile([_P, 8], f32, tag="bv8")
                    nc.vector.max(out=v8[:], in_=rsc[:])
                    p8 = pb.tile([_P, 8], u32, tag="bp8")
                    nc.vector.max_index(p8[:], v8[:], rsc[:])
                    p8f = pb.tile([_P, 8], f32, tag="bp8f")
                    nc.vector.tensor_copy(p8f[:], p8[:])
                    sel = pb.tile([_P, 8, _M], f32, tag="bsel")
                    nc.vector.tensor_tensor(
                        out=sel[:],
                        in0=p8f[:, :, None].to_broadcast([_P, 8, _M]),
                        in1=iota_t[:, None, 0:_M].to_broadcast([_P, 8, _M]),
                        op=EQ,
                    )
                    nc.vector.tensor_tensor(
                        out=sel[:],
                        in0=sel[:],
                        in1=cands_i[:, tt, None, 0:_M].to_broadcast([_P, 8, _M]),
                        op=MUL,
                    )
                    idf8 = pb.tile([_P, 8], f32, tag="idf8")
                    nc.vector.reduce_sum(idf8[:], sel[:], axis=AX)
                    ids32 = pb.tile([_P, 8], i32, tag="ids32")
                    nc.vector.tensor_copy(ids32[:], idf8[:])
                    nc.sync.dma_start(out_ids[tt * _P : (tt + 1) * _P, :], ids32[:])
                    idx8 = pb.tile([_P, 8], u32, tag="idx8")
                    nc.vector.tensor_copy(idx8[:], idf8[:])
                    G2 = pb.tile([_P, _K, _D], f32, tag="G2")
                    for j in range(_K):
                        nc.gpsimd.indirect_dma_start(
                            out=G2[:, j, :],
                            out_offset=None,
                            in_=cbaug[:],
                            in_offset=bass.IndirectOffsetOnAxis(
                                ap=idx8[:, j : j + 1], axis=0
                            ),
                        )
                    osum = pb.tile([_P, _D], f32, tag="osum")
                    nc.vector.reduce_sum(
                        osum[:], G2[:].rearrange("p k d -> p d k"), axis=AX
                    )
                    nc.scalar.mul(osum[:], osum[:], 1.0 / _K)
                    nc.sync.dma_start(out_x[tt * _P : (tt + 1) * _P, :], osum[:])

    nc.finalize()
    return nc


def _host_prep(x, cb, tt_n, cc_n):
    """Build per-core input maps. x: [ntok_total, D] fp32, cb: [ncodes, D]."""
    ncodes = cc_n * _CHUNK
    ntok = tt_n * _P
    n_cores = x.shape[0] // ntok
    c2 = (cb * cb).sum(axis=1, dtype=np.float32)

    # ct: [cc, kp, s, j]; s<8: cb[cc*CHUNK+j, s*128+kp]; s==8,kp==0: c2
    cbt = np.ascontiguousarray(cb.T)  # [D, ncodes]
    ct = np.zeros((cc_n, _P, _KSUB, _CHUNK), np.float32)
    # cbt.reshape(8, 128, cc_n, CHUNK)[s, kp, cc, j]
    ct[:, :, : _KSUB - 1, :] = cbt.reshape(_D // _P, _P, cc_n, _CHUNK).transpose(
        2, 1, 0, 3
    )
    ct[:, 0, _KSUB - 1, :] = c2.reshape(cc_n, _CHUNK)

    cbaug = np.zeros((ncodes, _AUGW), np.float32)
    cbaug[:, :_D] = cb
    cbaug[:, _D] = c2

    iota = np.broadcast_to(np.arange(32, dtype=np.float32), (_P, 32)).copy()

    in_maps = []
    for core in range(n_cores):
        xc = x[core * ntok : (core + 1) * ntok]  # [ntok, D]
        xt = np.zeros((_P, _KSUB, ntok), np.float32)
        # xt[kp, s, t] = 2*xc[t, s*128+kp] for s<8 ; s==8,kp==0 -> -1
        xt[:, : _KSUB - 1, :] = 2.0 * xc.T.reshape(_D // _P, _P, ntok).transpose(
            1, 0, 2
        )
        xt[0, _KSUB - 1, :] = -1.0
        xaug = np.zeros((_P, tt_n, _AUGW), np.float32)
        # xaug[p, tt, :1024] = 2*xc[tt*128+p], [..,1024] = -1
        xaug[:, :, :_D] = 2.0 * xc.reshape(tt_n, _P, _D).transpose(1, 0, 2)
        xaug[:, :, _D] = -1.0
        in_maps.append(
            {
                "xt": xt,
                "ct": ct,
                "cbaug": cbaug,
                "xaug": xaug,
                "iota": iota,
            }
        )
    return in_maps


def _fix_dup_rows(x, cb, c2, ids, outs):
    """Recompute tokens whose top-8 ids contain duplicates (exact-tie fallout)."""
    dup = np.zeros(ids.shape[0], bool)
    srt = np.sort(ids, axis=1)
    dup = (srt[:, 1:] == srt[:, :-1]).any(axis=1)
    if not dup.any():
        return 0
    rows = np.where(dup)[0]
    for t in rows:
        logits = 2.0 * (x[t] @ cb.T) - c2
        order = np.argsort(-logits, kind="stable")[:_K]
        ids[t] = order.astype(np.int32)
        outs[t] = cb[order].sum(axis=0, dtype=np.float32) / _K
    return len(rows)


def kernel(inputs, codebook, kcodes):
    from concourse.bass_utils import run_bass_kernel_spmd

    assert int(kcodes) == _K
    x = np.ascontiguousarray(np.asarray(inputs, np.float32).reshape(-1, _D))
    cb = np.ascontiguousarray(np.asarray(codebook, np.float32))
    tt_n, cc_n = _TPC // _P, _C // _CHUNK

    key = (tt_n, cc_n)
    if key not in _cache:
        _cache[key] = _build(tt_n, cc_n)
    nc = _cache[key]

    in_maps = _host_prep(x, cb, tt_n, cc_n)
    res = run_bass_kernel_spmd(nc, in_maps, list(range(_NCORES))).results

    outs = np.concatenate([r["out_x"] for r in res], axis=0)
    ids = np.concatenate([r["out_ids"] for r in res], axis=0)

    c2 = (cb * cb).sum(axis=1, dtype=np.float32)
    _fix_dup_rows(x, cb, c2, ids, outs)

    return (
        outs.reshape(_B, _S, _D),
        ids.reshape(_B, _S, _K).astype(np.int32),
    )
